# revision 1
# baseline (speedup 1.0000x reference)
"""CRF log-partition (forward algorithm) on 8 Trainium2 NeuronCores.

Math: the log-space scan  fv' = logsumexp_prev(fv + trans) + em_t  is run in
LINEAR space:  s' = (E @ s) * x_t  with E = exp(trans), x_t = exp(em_t - c_bt),
where c_bt = logsumexp_l(em[b,t,:]) is a host-side per-(b,t) prescale that keeps
all magnitudes in fp32 range (validated: state stays within [1e-7, 1e-2]).

Parallelism: batch is sharded 8 ways (64 b / core).  Serial depth is halved by
running the forward recursion for t=0..255 and the backward (beta) recursion
for t=511..256 simultaneously; they meet in the middle and are stitched with a
per-b dot product on the host.  On-chip, fwd and bwd are packed into one
128-partition scan: partitions = [fwd: l=0..63 | bwd: l=0..63], so each step is
ONE stationary-weight matmul (W = blockdiag(E^T, E)) + ONE VectorE multiply:

    S_{k+1} = (W^T-apply @ S_k) * X_k      (PSUM fp32 -> SBUF fp32)

The 64 batch elements per core are split into NCH independent chains (free-dim
columns) so PE/DVE pipeline across chains.  The host pre-packs X into the exact
[partition, slot*64+col] layout so the kernel DMAs contiguous slabs and does
zero on-chip transposes, exps, or renormalizations.
"""
import sys

import numpy as np

for _p in ("/opt/trn_rl_repo",):
    if _p not in sys.path:
        sys.path.insert(0, _p)

L = 64
START = L - 2
STOP = L - 1
B = 512
T = 512
NCORES = 8
BPC = B // NCORES      # 64 batch elements per core
Tm = T // 2            # 256 scan slots (fwd+bwd run simultaneously)
NCH = 2                # independent pipeline chains per core
J = BPC // NCH         # free-dim columns per chain
CHUNK = 32             # slots per X DMA chunk
NCHUNK = Tm // CHUNK

_cached = {}


def _build_bass():
    import concourse.bacc as bacc
    import concourse.mybir as mybir
    from concourse import tile

    f32 = mybir.dt.float32
    # Bacc (not bare Bass): its compile() runs move_matmul_waits_to_ldweights +
    # generate_event_semaphores, which split multi-sem waits to satisfy the
    # TRN2 1-wait-per-instruction ISA encoding limit.
    nc = bacc.Bacc()
    xd = nc.declare_dram_parameter("x", [128, Tm * 64], f32, isOutput=False)
    wd = nc.declare_dram_parameter("w", [128, 128], f32, isOutput=False)
    s0d = nc.declare_dram_parameter("s0", [128, BPC], f32, isOutput=False)
    outd = nc.declare_dram_parameter("out", [128, BPC], f32, isOutput=True)

    bf16 = mybir.dt.bfloat16
    with tile.TileContext(nc) as tc:
        with (
            tc.tile_pool(name="const", bufs=1) as cpool,
            tc.tile_pool(name="xbuf", bufs=1) as xpool,
            tc.tile_pool(name="state", bufs=4) as spool,
            tc.tile_pool(name="psum", bufs=3, space="PSUM") as ppool,
        ):
            # Stage w/s0 through DVE copies (f32 DRAM -> bf16 tiles): the
            # copies also absorb the two DMA queue semaphores so the first
            # matmul carries at most ONE sync wait (walrus LDWEIGHTS limit).
            # bf16 weights enable fast-weight-load on PE; bf16 state keeps
            # the matmul moving operand at 1 cycle/row.
            wraw = cpool.tile([128, 128], f32, name="wraw")
            nc.sync.dma_start(wraw[:], wd[:, :])
            s0raw = cpool.tile([128, BPC], f32, name="s0raw")
            nc.sync.dma_start(s0raw[:], s0d[:, :])
            w = cpool.tile([128, 128], bf16, name="w")
            nc.vector.tensor_copy(w[:], wraw[:])
            s0 = cpool.tile([128, BPC], bf16, name="s0")
            nc.vector.tensor_copy(s0[:], s0raw[:])

            xch = []
            for ci in range(NCHUNK):
                xt = xpool.tile([128, CHUNK * 64], f32, name=f"xc{ci}", tag=f"xc{ci}")
                nc.sync.dma_start(xt[:], xd[:, ci * CHUNK * 64:(ci + 1) * CHUNK * 64])
                # Absorb the chunk's DMA-queue semaphore into the DVE clock so
                # the steady-state muls stay within the 2-wait TT ISA limit.
                xab = cpool.tile([1, 1], f32, name=f"xab{ci}", tag="xab")
                nc.vector.tensor_copy(xab[:], xt[0:1, 0:1])
                xch.append(xt)

            state = [s0[:, g * J:(g + 1) * J] for g in range(NCH)]
            for k in range(Tm):
                ci, off = divmod(k, CHUNK)
                for g in range(NCH):
                    ps = ppool.tile([128, J], f32, name=f"ps{g}_{k}", tag=f"ps{g}")
                    nc.tensor.matmul(ps[:], lhsT=w[:], rhs=state[g], start=True, stop=True)
                    # Last slot writes f32 (the DRAM output dtype); steady
                    # state stays bf16 to keep the matmul fast.
                    odt = f32 if k == Tm - 1 else bf16
                    ns = spool.tile([128, J], odt, name=f"st{g}_{k}", tag=f"st{g}" if k < Tm - 1 else f"fin{g}")
                    xsl = xch[ci][:, off * 64 + g * J: off * 64 + (g + 1) * J]
                    nc.vector.tensor_mul(ns[:], ps[:], xsl)
                    state[g] = ns

            for g in range(NCH):
                nc.sync.dma_start(outd[:, g * J:(g + 1) * J], state[g])
    if not nc.is_finalized():
        nc.finalize()   # Bacc: runs wait-splitting + register allocation

    # The stationary weight matrix W never changes across the 512 matmuls, but
    # Bacc emits an InstLdweights before every InstMatmult (~230ns each on PE,
    # half of all PE time). Keep only the first load; the PE array retains the
    # weights across matmuls. (The removed LDWs carry no sync waits.)
    for blk in nc.m.functions[0].blocks:
        il = list(blk.instructions)
        keep, seen = [], 0
        for i in il:
            if type(i).__name__ == "InstLdweights":
                si = i.sync_info
                has_sync = si is not None and (len(si.on_wait) > 0 or len(si.on_update) > 0)
                seen += 1
                if seen > 1 and not has_sync:
                    continue
            keep.append(i)
        if len(keep) != len(il):
            blk.instructions = keep
    return nc


def _prepare_host(input, transitions):
    em = np.asarray(input, dtype=np.float32)          # [B,T,L]
    trans = np.asarray(transitions, dtype=np.float32)
    E = np.exp(trans.astype(np.float64))              # exp(-1e4) underflows to 0
    Ef = E.astype(np.float32)

    m = em.max(axis=2, keepdims=True)
    c = np.log(np.exp(em - m).sum(axis=2, keepdims=True)) + m   # [B,T,1] f32
    X = np.exp(em - c)                                          # [B,T,L] f32
    csum = c.astype(np.float64).sum(axis=(1, 2))                # [B]

    W = np.zeros((128, 128), np.float32)
    W[0:64, 0:64] = Ef.T        # fwd block: out_top = E @ S_top
    W[64:128, 64:128] = Ef      # bwd block: out_bot = E^T @ S_bot
    Estop = Ef[STOP, :]         # [64]

    in_maps = []
    for cidx in range(NCORES):
        Xc = X[cidx * BPC:(cidx + 1) * BPC]           # [64, T, L]  (b_local, t, l)
        XH = np.empty((Tm, 128, BPC), np.float32)     # [slot, partition, col=b_local]
        # fwd top half: slot k multiplies by x_{t=k}
        XH[:, 0:64, :] = Xc[:, 0:Tm, :].transpose(1, 2, 0)
        # bwd bottom half: slot k multiplies by x_{t=510-k}; slot 255 = ones
        tidx = 510 - np.arange(Tm - 1)
        XH[0:Tm - 1, 64:128, :] = Xc[:, tidx, :].transpose(1, 2, 0)
        XH[Tm - 1, 64:128, :] = 1.0
        xflat = np.ascontiguousarray(
            XH.transpose(1, 0, 2).reshape(128, Tm * BPC))

        s0 = np.zeros((128, BPC), np.float32)
        s0[START, :] = 1.0                            # fwd init: one-hot START
        s0[64:128, :] = (Xc[:, T - 1, :] * Estop).T   # bwd init: x_{511} * E[STOP,:]
        in_maps.append({"x": xflat, "w": W, "s0": s0})
    return in_maps, csum


def _stitch(results, csum):
    Z = np.empty(B, np.float64)
    for cidx in range(NCORES):
        out = results[cidx]["out"].astype(np.float64)   # [128, 64]
        dot = (out[0:64] * out[64:128]).sum(axis=0)     # [64] col = b_local
        Z[cidx * BPC:(cidx + 1) * BPC] = np.log(dot) + csum[cidx * BPC:(cidx + 1) * BPC]
    return Z.astype(np.float32)


def _enable_ldw_opt():
    """Flip walrus --enable-ldw-opt to true so the constant stationary weight
    matrix is loaded into the PE array once instead of per-matmul (the scan
    reuses one W for all 512 matmuls; the per-MM LDWEIGHTS otherwise costs
    ~230ns each)."""
    import os
    if os.environ.get("BASS_LDW_OPT") != "1":
        return   # default off: we de-dup LDWEIGHTS ourselves post-finalize
    from concourse import bass_utils
    if getattr(bass_utils.run_command, "_ldw_patched", False):
        return
    orig = bass_utils.run_command

    def patched(argv, **kwargs):
        argv = [a.replace("--enable-ldw-opt=false", "--enable-ldw-opt=true")
                if isinstance(a, str) else a for a in argv]
        return orig(argv, **kwargs)

    patched._ldw_patched = True
    bass_utils.run_command = patched


def _run(input, transitions, trace=False):
    _enable_ldw_opt()
    from concourse.bass_utils import run_bass_kernel_spmd

    if "nc" not in _cached:
        _cached["nc"] = _build_bass()
    nc = _cached["nc"]
    in_maps, csum = _prepare_host(input, transitions)
    res = run_bass_kernel_spmd(nc, in_maps, core_ids=list(range(NCORES)), trace=trace)
    return _stitch(res.results, csum), res


def kernel(input, transitions):
    out, _ = _run(input, transitions, trace=False)
    return out



# revision 2
# speedup vs baseline: 1.0183x; 1.0183x over previous
"""CRF log-partition (forward algorithm) on 8 Trainium2 NeuronCores.

Math: the log-space scan  fv' = logsumexp_prev(fv + trans) + em_t  is run in
LINEAR space:  s' = (E @ s) * x_t  with E = exp(trans), x_t = exp(em_t - c_bt),
where c_bt = logsumexp_l(em[b,t,:]) is a host-side per-(b,t) prescale that keeps
all magnitudes in fp32 range (validated: state stays within [1e-7, 1e-2]).

Parallelism: batch is sharded 8 ways (64 b / core).  Serial depth is halved by
running the forward recursion and the backward (beta) recursion simultaneously;
they meet in the middle and are stitched with a per-b bilinear form B^T E A on
the host.  On-chip, fwd and bwd are packed into one 128-partition scan:
partitions = [fwd: l=0..63 | bwd: l=0..63], so each step is ONE stationary-
weight matmul (W = blockdiag(E^T, E)) + ONE VectorE multiply:

    S_{k+1} = (W^T-apply @ S_k) * X_k      (PSUM fp32 -> SBUF bf16)

The 64 batch elements per core are split into NCH=2 independent chains
(free-dim columns) so the two chains' PE->DVE->PE round trips interleave: the
steady state runs at the single-chain round-trip latency (~467 ns = matmul
183 + sem 38 + mul 191 + sem 55), which is the hardware floor for this
dataflow (PE SBUF-access pipeline and DVE PSUM-access latency are fixed).

vs the earlier version: W / s0 / X are uploaded as bf16 directly from the
host (no on-chip staging casts; X DMA bytes halved), the first fwd step and
the final bwd E^T-hop are folded into the host prep/stitch (255 slots instead
of 256, no dummy ones-multiply), X streams in geometrically-growing chunks
(2,2,4,8,16,32,64,127 slots) so the first matmul isn't gated on a 1 MB DMA,
and chunk DMAs are enqueued from both the Sync and Scalar queues in parallel.
"""
import sys

import numpy as np

for _p in ("/opt/trn_rl_repo",):
    if _p not in sys.path:
        sys.path.insert(0, _p)

L = 64
START = L - 2
STOP = L - 1
B = 512
T = 512
NCORES = 8
BPC = B // NCORES      # 64 batch elements per core
Tm = 255               # scan slots (fwd+bwd run simultaneously; 2 steps folded to host)
NCH = 2                # independent pipeline chains per core
J = BPC // NCH         # free-dim columns per chain
CHUNKS = (2, 2, 4, 8, 16, 32, 64, 127)   # slots per X DMA chunk (sums to Tm)
assert sum(CHUNKS) == Tm

_cached = {}


def _build_bass():
    import concourse.bacc as bacc
    import concourse.mybir as mybir
    from concourse import tile

    f32 = mybir.dt.float32
    bf16 = mybir.dt.bfloat16
    # Bacc (not bare Bass): its compile() runs move_matmul_waits_to_ldweights +
    # generate_event_semaphores, which split multi-sem waits to satisfy the
    # TRN2 1-wait-per-instruction ISA encoding limit.
    nc = bacc.Bacc()
    xd = nc.declare_dram_parameter("x", [128, Tm * 64], bf16, isOutput=False)
    wd = nc.declare_dram_parameter("w", [128, 128], bf16, isOutput=False)
    s0d = nc.declare_dram_parameter("s0", [128, BPC], bf16, isOutput=False)
    outd = nc.declare_dram_parameter("out", [128, BPC], f32, isOutput=True)

    with tile.TileContext(nc) as tc:
        with (
            tc.tile_pool(name="const", bufs=1) as cpool,
            tc.tile_pool(name="xbuf", bufs=1) as xpool,
            tc.tile_pool(name="state", bufs=4) as spool,
            tc.tile_pool(name="psum", bufs=3, space="PSUM") as ppool,
        ):
            # w and s0 arrive pre-cast to bf16 from the host; the first
            # matmul's two DMA waits are legal because Bacc moves one onto
            # the ldweights instruction (move_matmul_waits_to_ldweights).
            w = cpool.tile([128, 128], bf16, name="w")
            nc.sync.dma_start(w[:], wd[:, :])
            s0 = cpool.tile([128, BPC], bf16, name="s0")
            nc.sync.dma_start(s0[:], s0d[:, :])

            # X streams in geometrically growing chunks: the first matmuls
            # only gate on a 32 KB transfer, while the bulk arrives under
            # the compute.  Chunks 0-1 enqueue on the Sync queue (already
            # issuing w/s0); the rest go through the otherwise-idle Scalar
            # queue so the enqueue cost (~0.6 us per DMA) overlaps.
            xch = []        # (tile, slot_offset, nslots)
            off = 0
            for ci, nsl in enumerate(CHUNKS):
                xt = xpool.tile([128, nsl * 64], bf16, name=f"xc{ci}", tag=f"xc{ci}")
                q = nc.sync if ci < 2 else nc.scalar
                q.dma_start(xt[:], xd[:, off * 64:(off + nsl) * 64])
                xch.append((xt, off, nsl))
                off += nsl

            # Final states of both chains land in one tile so a single DMA
            # writes the output.
            fin = spool.tile([128, BPC], f32, name="fin", tag="fin")

            state = [s0[:, g * J:(g + 1) * J] for g in range(NCH)]
            ci = 0
            for k in range(Tm):
                while k >= xch[ci][1] + xch[ci][2]:
                    ci += 1
                xt, coff, _ = xch[ci]
                off = k - coff
                for g in range(NCH):
                    ps = ppool.tile([128, J], f32, name=f"ps{g}_{k}", tag=f"ps{g}")
                    nc.tensor.matmul(ps[:], lhsT=w[:], rhs=state[g], start=True, stop=True)
                    xsl = xt[:, off * 64 + g * J: off * 64 + (g + 1) * J]
                    if k == Tm - 1:
                        # Last slot writes f32 into the shared output tile.
                        nc.vector.tensor_mul(fin[:, g * J:(g + 1) * J], ps[:], xsl)
                    else:
                        ns = spool.tile([128, J], bf16, name=f"st{g}_{k}", tag=f"st{g}")
                        nc.vector.tensor_mul(ns[:], ps[:], xsl)
                        state[g] = ns

            nc.sync.dma_start(outd[:, :], fin[:])
    if not nc.is_finalized():
        nc.finalize()   # Bacc: runs wait-splitting + register allocation

    # The stationary weight matrix W never changes across the matmuls, but
    # Bacc emits an InstLdweights before every InstMatmult (~230ns each on PE,
    # half of all PE time). Keep only the first load; the PE array retains the
    # weights across matmuls. (The removed LDWs carry no sync waits.)
    for blk in nc.m.functions[0].blocks:
        il = list(blk.instructions)
        keep, seen = [], 0
        for i in il:
            if type(i).__name__ == "InstLdweights":
                si = i.sync_info
                has_sync = si is not None and (len(si.on_wait) > 0 or len(si.on_update) > 0)
                seen += 1
                if seen > 1 and not has_sync:
                    continue
            keep.append(i)
        if len(keep) != len(il):
            blk.instructions = keep
    return nc


def _prepare_host(input, transitions):
    import ml_dtypes

    bf16 = ml_dtypes.bfloat16
    em = np.asarray(input, dtype=np.float32)          # [B,T,L]
    trans = np.asarray(transitions, dtype=np.float32)
    E = np.exp(trans.astype(np.float64))              # exp(-1e4) underflows to 0
    Ef = E.astype(np.float32)

    m = em.max(axis=2, keepdims=True)
    c = np.log(np.exp(em - m).sum(axis=2, keepdims=True)) + m   # [B,T,1] f32
    X = np.exp(em - c)                                          # [B,T,L] f32
    csum = c.astype(np.float64).sum(axis=(1, 2))                # [B]

    W = np.zeros((128, 128), np.float32)
    W[0:64, 0:64] = Ef.T        # fwd block: out_top = E @ S_top
    W[64:128, 64:128] = Ef      # bwd block: out_bot = E^T @ S_bot
    Estop = Ef[STOP, :]         # [64]
    Estart = Ef[:, START]       # [64] = E @ e_START

    in_maps = []
    for cidx in range(NCORES):
        Xc = X[cidx * BPC:(cidx + 1) * BPC]           # [64, T, L]  (b_local, t, l)
        XH = np.empty((Tm, 128, BPC), np.float32)     # [slot, partition, col=b_local]
        # fwd top half: slot k multiplies by x_{t=k+1} (step 0 folded into s0)
        XH[:, 0:64, :] = Xc[:, 1:Tm + 1, :].transpose(1, 2, 0)
        # bwd bottom half: slot k multiplies by x_{t=510-k} (x_511 in s0; the
        # final E^T hop is folded into the host stitch)
        tidx = 510 - np.arange(Tm)
        XH[:, 64:128, :] = Xc[:, tidx, :].transpose(1, 2, 0)
        xflat = np.ascontiguousarray(
            XH.transpose(1, 0, 2).reshape(128, Tm * BPC)).astype(bf16)

        s0 = np.zeros((128, BPC), np.float32)
        s0[0:64, :] = (Xc[:, 0, :] * Estart).T        # fwd init: x_0 * E[:,START]
        s0[64:128, :] = (Xc[:, T - 1, :] * Estop).T   # bwd init: x_511 * E[STOP,:]
        in_maps.append({"x": xflat, "w": W.astype(bf16), "s0": s0.astype(bf16)})
    return in_maps, csum, Ef


def _stitch(results, csum, Ef):
    E64 = Ef.astype(np.float64)
    Z = np.empty(B, np.float64)
    for cidx in range(NCORES):
        out = results[cidx]["out"].astype(np.float64)   # [128, 64]
        A = out[0:64]                                   # fwd alpha_255, col = b_local
        Bv = out[64:128]                                # bwd chain after 255 slots
        # Z_b = Bv_b^T E A_b  (the folded final E^T hop)
        dot = (Bv * (E64 @ A)).sum(axis=0)              # [64]
        Z[cidx * BPC:(cidx + 1) * BPC] = np.log(dot) + csum[cidx * BPC:(cidx + 1) * BPC]
    return Z.astype(np.float32)


def _run(input, transitions, trace=False):
    from concourse.bass_utils import run_bass_kernel_spmd

    if "nc" not in _cached:
        _cached["nc"] = _build_bass()
    nc = _cached["nc"]
    in_maps, csum, Ef = _prepare_host(input, transitions)
    res = run_bass_kernel_spmd(nc, in_maps, core_ids=list(range(NCORES)), trace=trace)
    return _stitch(res.results, csum, Ef), res


def kernel(input, transitions):
    out, _ = _run(input, transitions, trace=False)
    return out


# revision 3
# speedup vs baseline: 1.0322x; 1.0136x over previous
"""CRF log-partition (forward algorithm) on 8 Trainium2 NeuronCores.

Math: the log-space scan  fv' = logsumexp_prev(fv + trans) + em_t  is run in
LINEAR space:  s' = (E @ s) * x_t  with E = exp(trans), x_t = exp(em_t - c_bt),
where c_bt = logsumexp_l(em[b,t,:]) is a host-side per-(b,t) prescale that keeps
all magnitudes in fp32 range (validated: state stays within [1e-7, 1e-2]).

Parallelism: batch is sharded 8 ways (64 b / core).  Serial depth is halved by
running the forward recursion and the backward (beta) recursion simultaneously;
they meet in the middle and are stitched with a per-b bilinear form B^T E A on
the host.  On-chip, fwd and bwd are packed into one 128-partition scan:
partitions = [fwd: l=0..63 | bwd: l=0..63], so each step is ONE stationary-
weight matmul (W = blockdiag(E^T, E)) + ONE VectorE multiply:

    S_{k+1} = (W^T-apply @ S_k) * X_k      (PSUM fp32 -> SBUF bf16)

The 64 batch elements per core are split into NCH=2 independent chains
(free-dim columns) so the two chains' PE->DVE->PE round trips interleave: the
steady state runs at the single-chain round-trip latency (~467 ns = matmul
183 + sem 38 + mul 191 + sem 55), which is the hardware floor for this
dataflow (PE SBUF-access pipeline and DVE PSUM-access latency are fixed).

vs the earlier version: W / s0 / X are uploaded as bf16 directly from the
host (no on-chip staging casts; X DMA bytes halved), the first fwd step and
the final bwd E^T-hop are folded into the host prep/stitch (255 slots instead
of 256, no dummy ones-multiply), X streams in geometrically-growing chunks
(2,2,4,8,16,32,64,127 slots) so the first matmul isn't gated on a 1 MB DMA,
and chunk DMAs are enqueued from both the Sync and Scalar queues in parallel.
"""
import sys

import numpy as np

for _p in ("/opt/trn_rl_repo",):
    if _p not in sys.path:
        sys.path.insert(0, _p)

L = 64
START = L - 2
STOP = L - 1
B = 512
T = 512
NCORES = 8
BPC = B // NCORES      # 64 batch elements per core
Tm = 255               # scan slots (fwd+bwd run simultaneously; 2 steps folded to host)
NCH = 2                # independent pipeline chains per core
J = BPC // NCH         # free-dim columns per chain
CHUNKS = (2, 2, 4, 8, 16, 32, 64, 127)   # slots per X DMA chunk (sums to Tm)
assert sum(CHUNKS) == Tm

_cached = {}


def _build_bass():
    import concourse.bacc as bacc
    import concourse.mybir as mybir
    from concourse import tile

    f32 = mybir.dt.float32
    bf16 = mybir.dt.bfloat16
    # Bacc (not bare Bass): its compile() runs move_matmul_waits_to_ldweights +
    # generate_event_semaphores, which split multi-sem waits to satisfy the
    # TRN2 1-wait-per-instruction ISA encoding limit.
    nc = bacc.Bacc()
    xd = nc.declare_dram_parameter("x", [128, Tm * 64], bf16, isOutput=False)
    wd = nc.declare_dram_parameter("w", [128, 128], bf16, isOutput=False)
    s0d = nc.declare_dram_parameter("s0", [128, BPC], bf16, isOutput=False)
    outd = nc.declare_dram_parameter("out", [128, BPC], f32, isOutput=True)

    with tile.TileContext(nc) as tc:
        with (
            tc.tile_pool(name="const", bufs=1) as cpool,
            tc.tile_pool(name="xbuf", bufs=1) as xpool,
            tc.tile_pool(name="state", bufs=4) as spool,
            tc.tile_pool(name="psum", bufs=3, space="PSUM") as ppool,
        ):
            # w and s0 arrive pre-cast to bf16 from the host; the first
            # matmul's two DMA waits are legal because Bacc moves one onto
            # the ldweights instruction (move_matmul_waits_to_ldweights).
            w = cpool.tile([128, 128], bf16, name="w")
            nc.sync.dma_start(w[:], wd[:, :])
            s0 = cpool.tile([128, BPC], bf16, name="s0")
            nc.sync.dma_start(s0[:], s0d[:, :])

            # X streams in geometrically growing chunks: the first matmuls
            # only gate on a 32 KB transfer, while the bulk arrives under
            # the compute.  Chunks 0-1 enqueue on the Sync queue (already
            # issuing w/s0); the rest go through the otherwise-idle Scalar
            # queue so the enqueue cost (~0.6 us per DMA) overlaps.
            xch = []        # (tile, slot_offset, nslots)
            off = 0
            for ci, nsl in enumerate(CHUNKS):
                xt = xpool.tile([128, nsl * 64], bf16, name=f"xc{ci}", tag=f"xc{ci}")
                nc.scalar.dma_start(xt[:], xd[:, off * 64:(off + nsl) * 64])
                xch.append((xt, off, nsl))
                off += nsl

            # Final states of both chains land in one tile so a single DMA
            # writes the output.
            fin = spool.tile([128, BPC], f32, name="fin", tag="fin")

            state = [s0[:, g * J:(g + 1) * J] for g in range(NCH)]
            ci = 0
            for k in range(Tm):
                while k >= xch[ci][1] + xch[ci][2]:
                    ci += 1
                xt, coff, _ = xch[ci]
                off = k - coff
                for g in range(NCH):
                    ps = ppool.tile([128, J], f32, name=f"ps{g}_{k}", tag=f"ps{g}")
                    nc.tensor.matmul(ps[:], lhsT=w[:], rhs=state[g], start=True, stop=True)
                    xsl = xt[:, off * 64 + g * J: off * 64 + (g + 1) * J]
                    if k == Tm - 1:
                        # Last slot writes f32 into the shared output tile.
                        nc.vector.tensor_mul(fin[:, g * J:(g + 1) * J], ps[:], xsl)
                    else:
                        ns = spool.tile([128, J], bf16, name=f"st{g}_{k}", tag=f"st{g}")
                        nc.vector.tensor_mul(ns[:], ps[:], xsl)
                        state[g] = ns

            nc.sync.dma_start(outd[:, :], fin[:])
    if not nc.is_finalized():
        nc.finalize()   # Bacc: runs wait-splitting + register allocation

    # The stationary weight matrix W never changes across the matmuls, but
    # Bacc emits an InstLdweights before every InstMatmult (~230ns each on PE,
    # half of all PE time). Keep only the first load; the PE array retains the
    # weights across matmuls. (The removed LDWs carry no sync waits.)
    for blk in nc.m.functions[0].blocks:
        il = list(blk.instructions)
        keep, seen = [], 0
        for i in il:
            if type(i).__name__ == "InstLdweights":
                si = i.sync_info
                has_sync = si is not None and (len(si.on_wait) > 0 or len(si.on_update) > 0)
                seen += 1
                if seen > 1 and not has_sync:
                    continue
            keep.append(i)
        if len(keep) != len(il):
            blk.instructions = keep
    return nc


def _prepare_host(input, transitions):
    import ml_dtypes

    bf16 = ml_dtypes.bfloat16
    em = np.asarray(input, dtype=np.float32)          # [B,T,L]
    trans = np.asarray(transitions, dtype=np.float32)
    E = np.exp(trans.astype(np.float64))              # exp(-1e4) underflows to 0
    Ef = E.astype(np.float32)

    m = em.max(axis=2, keepdims=True)
    c = np.log(np.exp(em - m).sum(axis=2, keepdims=True)) + m   # [B,T,1] f32
    X = np.exp(em - c)                                          # [B,T,L] f32
    csum = c.astype(np.float64).sum(axis=(1, 2))                # [B]

    W = np.zeros((128, 128), np.float32)
    W[0:64, 0:64] = Ef.T        # fwd block: out_top = E @ S_top
    W[64:128, 64:128] = Ef      # bwd block: out_bot = E^T @ S_bot
    Estop = Ef[STOP, :]         # [64]
    Estart = Ef[:, START]       # [64] = E @ e_START

    in_maps = []
    for cidx in range(NCORES):
        Xc = X[cidx * BPC:(cidx + 1) * BPC]           # [64, T, L]  (b_local, t, l)
        XH = np.empty((Tm, 128, BPC), np.float32)     # [slot, partition, col=b_local]
        # fwd top half: slot k multiplies by x_{t=k+1} (step 0 folded into s0)
        XH[:, 0:64, :] = Xc[:, 1:Tm + 1, :].transpose(1, 2, 0)
        # bwd bottom half: slot k multiplies by x_{t=510-k} (x_511 in s0; the
        # final E^T hop is folded into the host stitch)
        tidx = 510 - np.arange(Tm)
        XH[:, 64:128, :] = Xc[:, tidx, :].transpose(1, 2, 0)
        xflat = np.ascontiguousarray(
            XH.transpose(1, 0, 2).reshape(128, Tm * BPC)).astype(bf16)

        s0 = np.zeros((128, BPC), np.float32)
        s0[0:64, :] = (Xc[:, 0, :] * Estart).T        # fwd init: x_0 * E[:,START]
        s0[64:128, :] = (Xc[:, T - 1, :] * Estop).T   # bwd init: x_511 * E[STOP,:]
        in_maps.append({"x": xflat, "w": W.astype(bf16), "s0": s0.astype(bf16)})
    return in_maps, csum, Ef


def _stitch(results, csum, Ef):
    E64 = Ef.astype(np.float64)
    Z = np.empty(B, np.float64)
    for cidx in range(NCORES):
        out = results[cidx]["out"].astype(np.float64)   # [128, 64]
        A = out[0:64]                                   # fwd alpha_255, col = b_local
        Bv = out[64:128]                                # bwd chain after 255 slots
        # Z_b = Bv_b^T E A_b  (the folded final E^T hop)
        dot = (Bv * (E64 @ A)).sum(axis=0)              # [64]
        Z[cidx * BPC:(cidx + 1) * BPC] = np.log(dot) + csum[cidx * BPC:(cidx + 1) * BPC]
    return Z.astype(np.float32)


def _run(input, transitions, trace=False):
    from concourse.bass_utils import run_bass_kernel_spmd

    if "nc" not in _cached:
        _cached["nc"] = _build_bass()
    nc = _cached["nc"]
    in_maps, csum, Ef = _prepare_host(input, transitions)
    res = run_bass_kernel_spmd(nc, in_maps, core_ids=list(range(NCORES)), trace=trace)
    return _stitch(res.results, csum, Ef), res


def kernel(input, transitions):
    out, _ = _run(input, transitions, trace=False)
    return out
